# revision 47
# baseline (speedup 1.0000x reference)
"""ALiBi multi-head causal attention on 8 TRN2 NeuronCores.

Sharding: core = b*4 + hg  (b in 0..1 batches, hg in 0..3 head-groups).
Each core computes 4 heads of one batch end-to-end (KQV projection for its
head-columns + causal ALiBi attention).  No collectives needed.

Per-core kernel (all matmuls bf16, f32 accumulation):
  - xT   [D, S]   : x[b].T -- contraction dim D on partitions
  - kqT  = (x W_kq)^T computed as [hd, S] per head (head-dim on partitions)
  - v    = x W_v computed as [S, hd] row-blocks (1/sqrt(hd) folded into q)
  - attention runs per head in groups of 4 query-blocks (512 columns) and
    works entirely in TRANSPOSED score space scoreT[t, sq] (k stationary,
    q-group moving), so the PV matmul consumes probsT directly and no
    per-chunk transposes exist anywhere:
      probsT[t,sq] = exp(scoreT + m*(tl-sqg) [+ causal -1e30] + shift(c-4G))
    No softmax max-subtraction (exponent <= score <= O(10) since the alibi
    bias is <= 0 in the causal region); far-past underflow to 0 is exact.
  - rowsum over t (partition axis) via M=1 ones-matmuls accumulated in PSUM;
    1/rowsum broadcast across partitions via a K=1 f32r matmul and
    reciprocal_approx_fast; out = (probsT-PV) * bcast(1/rowsum) + b_v.
  - output written as outT [head, hd, s]; host transposes back.
"""

import sys

if "/opt/trn_rl_repo" not in sys.path:
    sys.path.insert(0, "/opt/trn_rl_repo")

import numpy as np
import ml_dtypes

import concourse.bass as bass
import concourse.mybir as mybir
from concourse import bacc
from concourse.tile import TileContext
from concourse.bass_utils import run_bass_kernel_spmd

P = 128
S = 2048
D = 2048
HD = 128
NB = S // P            # 16 seq blocks
H_LOC = 4              # heads per core
NUM_HEADS = 16
SCALE = 1.0 / np.sqrt(HD)

F32 = mybir.dt.float32
F32R = mybir.dt.float32r
BF16 = mybir.dt.bfloat16
AF = mybir.ActivationFunctionType
OP = mybir.AluOpType


def _alibi_slopes(num_heads=NUM_HEADS):
    base = (2.0 ** 8) ** (1.0 / num_heads)
    return np.asarray([1.0 / base ** (i + 1) for i in range(num_heads)], np.float32)


def build():
    nc = bacc.Bacc("TRN2", target_bir_lowering=False)

    xT_d = nc.declare_dram_parameter("xT", [D, S], BF16, isOutput=False)
    wKQ_d = nc.declare_dram_parameter("wKQ", [D, 8 * P], BF16, isOutput=False)
    wV_d = nc.declare_dram_parameter("wV", [D, H_LOC * HD], BF16, isOutput=False)
    bKQ_d = nc.declare_dram_parameter("bKQ", [P, 8], F32, isOutput=False)
    bVT_d = nc.declare_dram_parameter("bVT", [HD, H_LOC], F32, isOutput=False)
    # transposed-space bias merged with causal mask variants:
    # biasT[j, 0] = m_j*(tl-sqg); biasT[j, 1+d] additionally has -1e30 where
    # tl > sql inside diagonal block d (d = chunk - 4G in 0..3)
    biasT_d = nc.declare_dram_parameter(
        "biasT", [H_LOC, 5, P, 512], F32, isOutput=False
    )
    # per-chunk shift: negshT[p, j, d+12] = m_j * 128 * d   (d = c - 4G)
    negshT_d = nc.declare_dram_parameter("negshT", [P, H_LOC, 16], F32, isOutput=False)
    # out in transposed-per-head layout [head, hd, s]; host transposes back
    out_d = nc.declare_dram_parameter("out", [H_LOC, HD, S], F32, isOutput=True)

    xT_t = xT_d.rearrange("(ko p) s -> p ko s", p=P)     # [128, 16, 2048]
    wKQ_t = wKQ_d.rearrange("(ko p) n -> p ko n", p=P)   # [128, 16, 1024]
    wV_t = wV_d.rearrange("(ko p) n -> p ko n", p=P)     # [128, 16, 512]

    with TileContext(nc) as tc:
        with (
            tc.tile_pool(name="const", bufs=1) as const,
            tc.tile_pool(name="resid", bufs=1) as resid,
            tc.tile_pool(name="stats", bufs=4) as stats,
            tc.tile_pool(name="psA", bufs=3, space="PSUM") as psA,
            tc.tile_pool(name="psO", bufs=2, space="PSUM") as psO,
            tc.tile_pool(name="psS", bufs=2, space="PSUM") as psS,
            tc.tile_pool(name="wpool", bufs=1) as wpool,
            tc.tile_pool(name="xpool", bufs=2) as xpool,
            tc.tile_pool(name="attn", bufs=2) as attn_pool,
            tc.tile_pool(name="biasp", bufs=2) as bias_pool,
        ):
            # ---- constants ----
            bkq_sb = const.tile([P, 8], F32)
            nc.sync.dma_start(bkq_sb, bKQ_d[:])
            bvt_sb = const.tile([HD, H_LOC], F32)
            nc.sync.dma_start(bvt_sb, bVT_d[:])

            negshT = const.tile([P, H_LOC, 16], F32)
            nc.sync.dma_start(negshT, negshT_d[:])

            ones_bf = const.tile([P, 1], BF16)  # rowsum column
            nc.gpsimd.memset(ones_bf, 1.0)
            ones1_raw = const.tile([1, P], F32)
            nc.gpsimd.memset(ones1_raw, 1.0)
            ones1_f = const.tile([1, P], F32R)  # partition-broadcast row
            with nc.allow_low_precision(reason="constant ones cast to f32r"):
                nc.vector.tensor_copy(ones1_f, ones1_raw)

            # ---- residents ----
            kq_all = resid.tile([P, 8, S], BF16)       # [hd, (K h0..3 | Q h0..3), s]
            v_all = resid.tile([P, NB, H_LOC * HD], BF16)  # [si, so, j*128+d]

            # ---- phase 1: KQV projection ----
            wkq_sb = wpool.tile([P, 16, 8 * P], BF16)
            for kk in range(4):
                nc.sync.dma_start(
                    wkq_sb[:, 4 * kk : 4 * kk + 4, :],
                    wKQ_t[:, 4 * kk : 4 * kk + 4, :],
                )
            wv_sb = wpool.tile([P, 16, H_LOC * HD], BF16)
            nc.sync.dma_start(wv_sb, wV_t)

            for nb in range(S // 512):
                xc = xpool.tile([P, 16, 512], BF16, tag="xc")
                nc.sync.dma_start(xc, xT_t[:, :, nb * 512 : (nb + 1) * 512])
                for m in range(8):
                    ps = psA.tile([P, 512], F32, tag="ps")
                    for k in range(16):
                        nc.tensor.matmul(
                            ps,
                            lhsT=wkq_sb[:, k, m * P : (m + 1) * P],
                            rhs=xc[:, k, :],
                            start=(k == 0),
                            stop=(k == 15),
                        )
                    # kqT = psum * scale + bias (scale folds 1/sqrt(hd) into q)
                    nc.scalar.activation(
                        kq_all[:, m, nb * 512 : (nb + 1) * 512],
                        ps,
                        AF.Identity,
                        bias=bkq_sb[:, m : m + 1],
                        scale=float(SCALE) if m >= 4 else 1.0,
                    )
                for sub in range(4):
                    s_idx = nb * 4 + sub
                    psv = psA.tile([P, 512], F32, tag="ps")
                    for k in range(16):
                        nc.tensor.matmul(
                            psv,
                            lhsT=xc[:, k, sub * P : (sub + 1) * P],
                            rhs=wv_sb[:, k, :],
                            start=(k == 0),
                            stop=(k == 15),
                        )
                    nc.vector.tensor_copy(v_all[:, s_idx, :], psv)

            # ---- phase 2: attention, transposed score space ----
            # scoreT[t, sq]: k stationary, q-group moving (N=512).  Softmax
            # needs only elementwise ops (bias/mask/exp) + a partition-axis
            # rowsum (M=1 ones-matmul).  PV consumes probsT directly -- no
            # per-chunk transposes anywhere.  Only the causally-valid column
            # range [lo:512] of each chunk is computed; the rest is zeroed.
            for j in range(H_LOC):
                biasT = bias_pool.tile([P, 5, 512], F32, tag="biasT")
                nc.sync.dma_start(biasT, biasT_d[j].rearrange("v p s -> p v s"))
                for G in range(NB // 4):
                    last_c = 4 * G + 3
                    # probsT[t, c, group_col]
                    probsT = attn_pool.tile([P, NB, 512], BF16, tag="pT")
                    rs_ps = psS.tile([1, 512], F32, tag="rs")
                    for c in range(last_c + 1):
                        d = c - 4 * G  # -12..3
                        lo = max(0, d) * P  # first causally-valid column
                        if lo > 0:
                            nc.vector.memset(probsT[:, c, :lo], 0.0)
                        w = 512 - lo
                        ps = psA.tile([P, 512], F32, tag="ps")
                        nc.tensor.matmul(
                            ps[:, :w],
                            lhsT=kq_all[:, j, c * P : (c + 1) * P],
                            rhs=kq_all[:, 4 + j, G * 512 + lo : (G + 1) * 512],
                            start=True,
                            stop=True,
                        )
                        v_idx = 1 + d if d >= 0 else 0
                        scoreT = attn_pool.tile([P, 512], F32, tag="scT")
                        nc.vector.tensor_tensor(
                            scoreT[:, lo:], ps[:, :w], biasT[:, v_idx, lo:], OP.add
                        )
                        nc.scalar.activation(
                            probsT[:, c, lo:],
                            scoreT[:, lo:],
                            AF.Exp,
                            bias=negshT[:, j, d + 12 : d + 13],
                            scale=1.0,
                        )
                        nc.tensor.matmul(
                            rs_ps,
                            lhsT=ones_bf,
                            rhs=probsT[:, c, :],
                            start=(c == 0),
                            stop=(c == last_c),
                        )
                    # PV: outT[hd, sq_group] accumulated over t-chunks
                    po = psO.tile([P, 512], F32, tag="po")
                    for c in range(last_c + 1):
                        nc.tensor.matmul(
                            po,
                            lhsT=v_all[:, c, j * HD : (j + 1) * HD],
                            rhs=probsT[:, c, :],
                            start=(c == 0),
                            stop=(c == last_c),
                        )
                    # normalize: reciprocal of rowsum, broadcast across
                    # partitions on GpSimd, then one DVE multiply
                    rs_sb = stats.tile([1, 512], F32R, tag="rs_sb")
                    with nc.allow_low_precision(reason="f32r rounding only"):
                        nc.vector.reciprocal(rs_sb, rs_ps)
                    rb = psS.tile([P, 512], F32, tag="rb", bufs=1)
                    nc.tensor.matmul(
                        rb, lhsT=ones1_f, rhs=rs_sb, start=True, stop=True
                    )
                    rb_sb = attn_pool.tile([P, 512], F32, tag="rbsb")
                    nc.vector.tensor_copy(rb_sb, rb)
                    out_sb = attn_pool.tile([P, 512], F32, tag="osb")
                    nc.vector.tensor_tensor(out_sb, po, rb_sb, OP.mult)
                    # + V-projection bias (sum of normalized probs == 1)
                    nc.scalar.activation(
                        out_sb,
                        out_sb,
                        AF.Identity,
                        bias=bvt_sb[:, j : j + 1],
                        scale=1.0,
                    )
                    nc.sync.dma_start(
                        out_d[j][:, G * 512 : (G + 1) * 512], out_sb
                    )

    nc.finalize()
    return nc


_NC_CACHE = None


def _get_nc():
    global _NC_CACHE
    if _NC_CACHE is None:
        _NC_CACHE = build()
    return _NC_CACHE


def _make_in_maps(x, W_kqv, b_kqv):
    x = np.asarray(x, np.float32)
    W = np.asarray(W_kqv, np.float32)
    b = np.asarray(b_kqv, np.float32)
    slopes = _alibi_slopes()
    in_maps = []
    for core in range(8):
        bi, hg = divmod(core, 4)
        heads = [4 * hg + j for j in range(H_LOC)]
        xT = np.ascontiguousarray(x[bi].T).astype(ml_dtypes.bfloat16)
        wkq = np.concatenate(
            [W[:, h * HD : (h + 1) * HD] for h in heads]
            + [W[:, D + h * HD : D + (h + 1) * HD] for h in heads],
            axis=1,
        ).astype(ml_dtypes.bfloat16)
        wv = np.concatenate(
            [W[:, 2 * D + h * HD : 2 * D + (h + 1) * HD] for h in heads], axis=1
        ).astype(ml_dtypes.bfloat16)
        # bias columns: K h0..h3 then Q h0..h3; q-side prescaled by 1/sqrt(hd)
        bkq = np.stack(
            [b[h * HD : (h + 1) * HD] for h in heads]
            + [b[D + h * HD : D + (h + 1) * HD] * SCALE for h in heads],
            axis=1,
        ).astype(np.float32)
        bvt = np.stack(
            [b[2 * D + h * HD : 2 * D + (h + 1) * HD] for h in heads], axis=1
        ).astype(np.float32)  # [hd, H_LOC]
        # biasT[j, v, tl, sqg]: v=0 plain m_j*(tl-sqg); v=1+d adds -1e30
        # where tl > sql inside diagonal block d
        relT = (np.arange(P)[:, None] - np.arange(512)[None, :]).astype(np.float32)
        base = slopes[heads][:, None, None] * relT[None]  # [4, 128, 512]
        causal_blk = np.where(
            np.arange(P)[:, None] > np.arange(P)[None, :], -1e30, 0.0
        ).astype(np.float32)
        bias_t = np.zeros((H_LOC, 5, P, 512), np.float32)
        bias_t[:, 0] = base
        for dd in range(4):
            v = base.copy()
            v[:, :, dd * P : (dd + 1) * P] += causal_blk[None]
            bias_t[:, 1 + dd] = v
        # negshT[p, j, d+12] = m_j * 128 * d, d in [-12, 3]
        dvals = (np.arange(16) - 12).astype(np.float32) * P
        negsht = np.tile(
            (slopes[heads][:, None] * dvals[None, :])[None], (P, 1, 1)
        ).astype(np.float32)
        in_maps.append(
            dict(
                xT=xT, wKQ=wkq, wV=wv, bKQ=bkq, bVT=bvt,
                biasT=bias_t, negshT=negsht,
            )
        )
    return in_maps


def run(inputs, trace=False, **kw):
    nc = _get_nc()
    in_maps = _make_in_maps(inputs["x"], inputs["W_kqv"], inputs["b_kqv"])
    bkr = run_bass_kernel_spmd(nc, in_maps, core_ids=list(range(8)), trace=trace, **kw)
    B = 2
    out = np.empty((B, NUM_HEADS, S, HD), np.float32)
    for core in range(8):
        bi, hg = divmod(core, 4)
        o = np.asarray(bkr.results[core]["out"])  # [4, 128(hd), 2048(s)]
        for j in range(H_LOC):
            out[bi, 4 * hg + j] = o[j].T
    return out, bkr


def kernel(x, W_kqv, b_kqv):
    out, _ = run({"x": x, "W_kqv": W_kqv, "b_kqv": b_kqv})
    return out
